# revision 1
# baseline (speedup 1.0000x reference)
"""CTC loss (keras ctc_batch_cost semantics) on 8 Trainium2 NeuronCores.

Self-contained: kernel(y_true, y_pred) -> loss [B, 1] float32.

Data-parallel over batch: 8 cores x 32 examples.  Per core:
  - Stream y_pred[e, t-half, :] tiles [128, 1000] fp32 from HBM (HWDGE).
  - gpsimd indirect_copy gathers the 129 extended-label class columns
    (+pad to 144) per t-row; index lists host-precomputed from y_true.
  - DVE adds eps; HWDGE SBUF->SBUF rearrange into G[example-part, t*SW+s].
  - CTC forward DP over T=256 steps in probability domain on DVE
    ([32, 132] fp32 tiles, 4 tensor ops/step) with sum-renormalization
    every 4 steps (tensor_tensor_reduce accum + reciprocal; scales cancel
    exactly via sum(ln inv) on the host).
  - Device returns raw [afin, inv_0..inv_63] per example; host computes
    loss = sum(ln inv_g, g<63) - ln(afin).
Falls back to a float64 numpy implementation if the device path fails.
"""

import numpy as np

EPS = 1e-7
B, T, C, L = 256, 256, 1000, 64
S = 2 * L + 1
NIDX = 144
SW = 132
AW = SW + 4
RENORM = 4
BLANK = C - 1
NCORES = 8
E = B // NCORES
TCHUNK = 128
NG = T // RENORM


def _build_ext(labels):
    ext = np.full(S, BLANK, dtype=np.int64)
    ext[1::2] = labels
    return ext


def _pack_idx(y_true):
    out = np.zeros((128, (NIDX // 16) * E), dtype=np.uint16)
    for e in range(E):
        ext = np.full(NIDX, BLANK, dtype=np.uint16)
        ext[:S] = _build_ext(y_true[e])
        wr = ext.reshape(NIDX // 16, 16)
        for g in range(8):
            out[16 * g:16 * g + 16,
                (NIDX // 16) * e:(NIDX // 16) * (e + 1)] = wr.T
    return out


def _pack_mask(y_true):
    M = np.zeros((E, SW), dtype=np.float32)
    for e in range(E):
        ext = _build_ext(y_true[e])
        prev2 = np.full(S, -1, dtype=np.int64)
        prev2[2:] = ext[:-2]
        M[e, :S] = ((ext != BLANK) & (ext != prev2)).astype(np.float32)
    return M


def _build_nc():
    import concourse.bacc as bacc
    import concourse.tile as tile
    from concourse import mybir

    F32 = mybir.dt.float32
    U16 = mybir.dt.uint16
    nchunk = T // TCHUNK
    nc = bacc.Bacc("TRN2", target_bir_lowering=False)

    pred_d = nc.dram_tensor("y_pred", [E, T, C], F32, kind="ExternalInput")
    idx_d = nc.dram_tensor("idxw", [128, (NIDX // 16) * E], U16,
                           kind="ExternalInput")
    mask_d = nc.dram_tensor("mask", [E, SW], F32, kind="ExternalInput")
    raw_d = nc.dram_tensor("raw", [E, 1 + NG], F32, kind="ExternalOutput")

    pred_rows = pred_d.rearrange("e t c -> (e t) c")

    with tile.TileContext(nc) as tc:
        with (
            tc.tile_pool(name="const", bufs=1) as constp,
            tc.tile_pool(name="pred", bufs=4) as predp,
            tc.tile_pool(name="gath", bufs=4) as gathp,
            tc.tile_pool(name="gbf", bufs=4) as gbfp,
        ):
            idx_sb = []
            for e in range(E):
                ie = constp.tile([128, NIDX // 16], U16, tag=f"idx{e}",
                                 name=f"idx{e}")
                nc.sync.dma_start(
                    ie[:, :],
                    idx_d[:, (NIDX // 16) * e:(NIDX // 16) * (e + 1)])
                idx_sb.append(ie)
            mask_sb = constp.tile([E, SW], F32, tag="mask", name="mask_sb")
            nc.sync.dma_start(mask_sb[:, :], mask_d[:, :])

            G = [constp.tile([E, TCHUNK * SW], F32, tag=f"G{c}", name=f"G{c}")
                 for c in range(nchunk)]
            alphaA = constp.tile([E, AW], F32, tag="alphaA", name="alphaA")
            alphaB = constp.tile([E, AW], F32, tag="alphaB", name="alphaB")
            albufs = [alphaA, alphaB]
            v = constp.tile([E, SW], F32, tag="v", name="v")
            u = constp.tile([E, SW], F32, tag="u", name="u")
            w = constp.tile([E, SW], F32, tag="w", name="w")
            sums_cur = constp.tile([E, 1], F32, tag="sc", name="sums_cur")
            inv_cur = constp.tile([E, 1], F32, tag="ic", name="inv_cur")
            inv = constp.tile([E, NG], F32, tag="inv", name="inv")
            raw_sb = constp.tile([E, 1 + NG], F32, tag="raw", name="raw_sb")

            nc.vector.memset(alphaA[:, :], 0.0)
            nc.vector.memset(alphaB[:, :], 0.0)

            def stream_chunk(c):
                for e in range(E):
                    pt = predp.tile([TCHUNK, C], F32, tag="pred", name="pt")
                    r0 = e * T + c * TCHUNK
                    nc.sync.dma_start(pt[:, :], pred_rows[r0:r0 + TCHUNK, :])
                    gt = gathp.tile([TCHUNK, NIDX], F32, tag="gath", name="gt")
                    nc.gpsimd.indirect_copy(
                        gt[:, :], pt[:, :], idx_sb[e][:, :], True)
                    gb = gbfp.tile([TCHUNK, SW], F32, tag="gbf", name="gb")
                    nc.vector.tensor_scalar_add(gb[:, :], gt[:, 0:SW], EPS)
                    dst = G[c][e:e + 1, :].rearrange(
                        "p (t s) -> p t s", t=TCHUNK)
                    nc.sync.dma_start(dst, gb[:, :])

            def dp_steps(c):
                for t in range(c * TCHUNK, (c + 1) * TCHUNK):
                    Gc = G[t // TCHUNK]
                    o = (t % TCHUNK) * SW
                    if t == 0:
                        nc.vector.tensor_copy(albufs[0][:, 2:4], G[0][:, 0:2])
                        continue
                    ap = albufs[(t - 1) % 2]
                    an = albufs[t % 2]
                    nc.vector.tensor_add(
                        v[:, 0:SW], ap[:, 2:2 + SW], ap[:, 1:1 + SW])
                    nc.vector.tensor_mul(
                        u[:, 0:SW], ap[:, 0:SW], mask_sb[:, 0:SW])
                    nc.vector.tensor_add(w[:, 0:SW], v[:, 0:SW], u[:, 0:SW])
                    g = t // RENORM
                    if t % RENORM == 0:
                        nc.vector.scalar_tensor_tensor(
                            an[:, 2:2 + SW], Gc[:, o:o + SW],
                            inv_cur[:, 0:1], w[:, 0:SW],
                            op0=mybir.AluOpType.mult,
                            op1=mybir.AluOpType.mult)
                    elif t % RENORM == RENORM - 1:
                        nc.vector.tensor_tensor_reduce(
                            an[:, 2:2 + SW], Gc[:, o:o + SW], w[:, 0:SW],
                            scale=1.0, scalar=0.0,
                            op0=mybir.AluOpType.mult,
                            op1=mybir.AluOpType.add,
                            accum_out=sums_cur[:, 0:1])
                        nc.vector.reciprocal(
                            inv_cur[:, 0:1], sums_cur[:, 0:1])
                        nc.vector.tensor_copy(
                            inv[:, g:g + 1], inv_cur[:, 0:1])
                    else:
                        nc.vector.tensor_mul(
                            an[:, 2:2 + SW], Gc[:, o:o + SW], w[:, 0:SW])

            for c in range(nchunk):
                stream_chunk(c)
                dp_steps(c)

            nc.vector.tensor_add(
                raw_sb[:, 0:1], albufs[(T - 1) % 2][:, 129:130],
                albufs[(T - 1) % 2][:, 130:131])
            nc.vector.tensor_copy(raw_sb[:, 1:1 + NG], inv[:, 0:NG])
            nc.sync.dma_start(raw_d[:, :], raw_sb[:, :])

    nc.compile()
    return nc


_NC_CACHE = {}


def _make_in_maps(y_true, y_pred):
    in_maps = []
    for k in range(NCORES):
        sl = slice(k * E, (k + 1) * E)
        yt = np.asarray(y_true[sl], dtype=np.int64)
        in_maps.append({
            "y_pred": np.ascontiguousarray(y_pred[sl]),
            "idxw": _pack_idx(yt),
            "mask": _pack_mask(yt),
        })
    return in_maps


def _finish(raw):
    """raw [E, 1+NG] -> loss [E, 1]; scales cancel exactly."""
    afin = raw[:, 0].astype(np.float64)
    inv = raw[:, 1:1 + NG].astype(np.float64)
    return (np.log(inv[:, :NG - 1]).sum(1) - np.log(afin))[:, None]


def _numpy_ctc(y_pred, y_true):
    Bn = y_pred.shape[0]
    NEGI = -1e30
    out = np.zeros((Bn, 1), dtype=np.float64)
    logp = np.log(y_pred.astype(np.float64) + EPS)
    for b in range(Bn):
        ext = _build_ext(y_true[b])
        lp = logp[b][:, ext]
        prev2 = np.full(S, -1, dtype=np.int64)
        prev2[2:] = ext[:-2]
        allow = (ext != BLANK) & (ext != prev2)
        al = np.full(S, NEGI)
        al[0], al[1] = lp[0, 0], lp[0, 1]
        for t in range(1, T):
            sh1 = np.concatenate(([NEGI], al[:-1]))
            sh2 = np.where(allow,
                           np.concatenate(([NEGI, NEGI], al[:-2])), NEGI)
            m = np.maximum(np.maximum(al, sh1), sh2)
            al = m + np.log(np.exp(al - m) + np.exp(sh1 - m)
                            + np.exp(sh2 - m)) + lp[t]
        m = max(al[S - 1], al[S - 2])
        out[b, 0] = -(m + np.log(np.exp(al[S - 1] - m)
                                 + np.exp(al[S - 2] - m)))
    return out


def kernel(y_true, y_pred):
    y_true = np.asarray(y_true)
    y_pred = np.ascontiguousarray(np.asarray(y_pred, dtype=np.float32))
    try:
        from concourse.bass_utils import run_bass_kernel_spmd
        if "nc" not in _NC_CACHE:
            _NC_CACHE["nc"] = _build_nc()
        res = run_bass_kernel_spmd(_NC_CACHE["nc"],
                                   _make_in_maps(y_true, y_pred),
                                   core_ids=list(range(NCORES)))
        loss = np.concatenate([_finish(r["raw"]) for r in res.results], 0)
        if not np.all(np.isfinite(loss)):
            raise FloatingPointError("non-finite loss from device")
        return loss.astype(np.float32)
    except Exception:
        return _numpy_ctc(y_pred, y_true).astype(np.float32)



# revision 11
# speedup vs baseline: 1.8778x; 1.8778x over previous
"""CTC loss (keras ctc_batch_cost semantics) on 8 Trainium2 NeuronCores.

Self-contained: kernel(y_true, y_pred) -> loss [B, 1] float32.

Data-parallel over batch: 8 cores x 32 examples.  Per core (v2.5):
  - Batched HWDGE DMA: pred t-chunks [Tc, 8 examples x 1008] fp32 (8
    zero-pad columns per example back the masked-skip gather slots).
  - One gpsimd indirect_copy per (example, chunk) gathers 288 columns:
    [0:132) = extended-symbol probs G, [144:276) = skip-masked probs MG
    (masked slots point at the zero columns).
  - ACT engine adds eps and converts to bf16; two SBUF->SBUF DMAs per
    chunk rearrange into DP layout G[e, t*132+s], MG[e, t*132+s].
  - CTC forward DP on DVE, [32, 132] bf16: per step
    v = a + shift1(a); q = shift2(a)*MG_t; p = v*G_t; a' = p + q
    with sum-renorm every 4 steps (TTR accum + reciprocal; scales cancel
    exactly via sum(ln inv) on the host).
  - t-chunks {64, 96, 96} shrink the pipeline warmup before step 0.
  - Device returns raw [afin, inv_0..inv_63]; host:
    loss = sum(ln inv_g, g<63) - ln(afin).
Falls back to a float64 numpy implementation if the device path fails.
"""

import numpy as np

EPS = 1e-7
B, T, C, L = 256, 256, 1000, 64
S = 2 * L + 1
SW = 132
CP = 1008                  # padded per-example column count (8 zero cols)
NIDX = 288                 # gather width: 132 G + pad + 132 MG + pad
IPC = NIDX // 16           # idx columns per example (18)
RENORM = 8
BLANK = C - 1
NCORES = 8
E = B // NCORES
CHUNKS = (128, 128)
NCHUNK = len(CHUNKS)
EG = 2                     # examples per pred DMA group
NGRP = E // EG
NG = T // RENORM


def _build_ext(labels):
    ext = np.full(S, BLANK, dtype=np.int64)
    ext[1::2] = labels
    return ext


def _pack_idx(y_true):
    """Per example: 288 u16 indices into the 1008-wide padded row.
    [0:132)   -> ext symbol columns (pad rows -> blank)
    [144:276) -> ext symbol columns where skip allowed, else zero-col.
    Wrapped per 16 partitions, replicated across the 8 gpsimd cores."""
    out = np.zeros((128, IPC * E), dtype=np.uint16)
    for e in range(E):
        ext = np.full(SW, BLANK, dtype=np.int64)
        ext[:S] = _build_ext(y_true[e])
        prev2 = np.full(S, -1, dtype=np.int64)
        prev2[2:] = ext[:S][:-2]
        allow = np.zeros(SW, dtype=bool)
        allow[:S] = (ext[:S] != BLANK) & (ext[:S] != prev2)
        idx = np.full(NIDX, 1000 + (BLANK % 8), dtype=np.uint16)
        idx[0:SW] = ext
        idx[144:144 + SW] = np.where(allow, ext, 1000 + (np.arange(SW) % 8))
        wr = idx.reshape(IPC, 16)
        for g in range(8):
            out[16 * g:16 * g + 16, IPC * e:IPC * (e + 1)] = wr.T
    return out


def _build_nc():
    import concourse.bacc as bacc
    import concourse.tile as tile
    from concourse import mybir

    F32 = mybir.dt.float32
    BF16 = mybir.dt.bfloat16
    U16 = mybir.dt.uint16
    MUL = mybir.AluOpType.mult
    ADD = mybir.AluOpType.add
    nc = bacc.Bacc("TRN2", target_bir_lowering=False)

    pred_d = nc.dram_tensor("y_pred", [E, T, C], F32, kind="ExternalInput")
    idx_d = nc.dram_tensor("idxw", [128, IPC * E], U16,
                           kind="ExternalInput")
    raw_d = nc.dram_tensor("raw", [E, 1 + NG], F32, kind="ExternalOutput")

    tbase = [0]
    for tc_ in CHUNKS:
        tbase.append(tbase[-1] + tc_)

    with tile.TileContext(nc) as tc:
        NPT = 4
        with (
            tc.tile_pool(name="const", bufs=1) as constp,
            tc.tile_pool(name="gath", bufs=4) as gathp,
            tc.tile_pool(name="gbf", bufs=2) as gbfp,
            tc.tile_pool(name="gpool", bufs=2) as gpoolp,
            tc.tile_pool(name="mpool", bufs=2) as mpoolp,
        ):
            idx_t = constp.tile([128, IPC * E], U16, tag="idx", name="idx_t")
            nc.sync.dma_start(idx_t[:, :], idx_d[:, :])

            # Persistent pred buffers with one-time zero-pad columns; the
            # DMAs only ever write the [0:C) block of each example slot, so
            # the pad stays zero across reuse.
            pt_bufs = [constp.tile([128, EG * CP], F32, tag=f"pt{i}",
                                   name=f"pt{i}") for i in range(NPT)]
            for i in range(NPT):
                zv = pt_bufs[i][:, :].rearrange("t (e c) -> t e c", e=EG)
                nc.vector.memset(zv[:, :, C:CP], 0.0)

            G = [gpoolp.tile([E, CHUNKS[c] * SW], BF16, tag="G",
                             name=f"G{c}") for c in range(NCHUNK)]
            MG = [mpoolp.tile([E, CHUNKS[c] * SW], BF16, tag="M",
                              name=f"M{c}") for c in range(NCHUNK)]
            alphaA = constp.tile([E, SW + 4], BF16, tag="alphaA",
                                 name="alphaA")
            alphaB = constp.tile([E, SW + 4], BF16, tag="alphaB",
                                 name="alphaB")
            albufs = [alphaA, alphaB]
            v = constp.tile([E, SW], BF16, tag="v", name="v")
            p = constp.tile([E, SW], BF16, tag="p", name="p")
            q = constp.tile([E, SW], BF16, tag="q", name="q")
            sums = constp.tile([E, 1], F32, tag="sc", name="sums")
            invb = constp.tile([E, NG], F32, tag="inv", name="invb")
            raw_sb = constp.tile([E, 1 + NG], F32, tag="raw", name="raw_sb")

            nc.vector.memset(alphaA[:, :], 0.0)
            nc.vector.memset(alphaB[:, :], 0.0)

            def stream_chunk(c):
                tc_ = CHUNKS[c]
                t0 = tbase[c]
                gb = gbfp.tile([tc_, E * 264], BF16, tag="gbuf", name="gb")
                for g in range(NGRP):
                    pt = pt_bufs[(c * NGRP + g) % NPT]
                    zv = pt[0:tc_, :].rearrange("t (e c) -> t e c", e=EG)
                    src = pred_d[EG * g:EG * (g + 1), t0:t0 + tc_, :]
                    dst = zv[:, :, 0:C].rearrange("t e c -> e t c")
                    nc.sync.dma_start(dst, src)
                    for el in range(EG):
                        e = EG * g + el
                        gt = gathp.tile([tc_, NIDX], F32, tag="gath",
                                        name="gt")
                        nc.gpsimd.indirect_copy(
                            gt[:, :], pt[0:tc_, el * CP:(el + 1) * CP],
                            idx_t[0:tc_, IPC * e:IPC * (e + 1)], True)
                        nc.scalar.activation(
                            gb[:, e * 264:e * 264 + SW], gt[:, 0:SW],
                            mybir.ActivationFunctionType.Copy, bias=EPS)
                        nc.scalar.activation(
                            gb[:, e * 264 + SW:e * 264 + 264],
                            gt[:, 144:144 + SW],
                            mybir.ActivationFunctionType.Copy, bias=EPS)
                srcg = gb[:, :].rearrange("t (e c) -> e t c", e=E)
                dstG = G[c][:, :].rearrange("e (t s) -> e t s", t=tc_)
                dstM = MG[c][:, :].rearrange("e (t s) -> e t s", t=tc_)
                th = tc_ // 2
                for lo, hi in ((0, th), (th, tc_)):
                    nc.sync.dma_start(dstG[:, lo:hi, :],
                                      srcg[:, lo:hi, 0:SW])
                    nc.sync.dma_start(dstM[:, lo:hi, :],
                                      srcg[:, lo:hi, SW:264])

            def dp_steps(c):
                tc_ = CHUNKS[c]
                t0 = tbase[c]
                for t in range(t0, t0 + tc_):
                    o = (t - t0) * SW
                    Gc, Mc = G[c], MG[c]
                    if t == 0:
                        nc.vector.tensor_copy(albufs[0][:, 2:4],
                                              G[0][:, 0:2])
                        continue
                    ap = albufs[(t - 1) % 2]
                    an = albufs[t % 2]
                    g = t // RENORM
                    nc.vector.tensor_add(
                        v[:, :], ap[:, 2:2 + SW], ap[:, 1:1 + SW])
                    if t % RENORM == 0:
                        nc.vector.scalar_tensor_tensor(
                            q[:, :], ap[:, 0:SW], invb[:, g - 1:g],
                            Mc[:, o:o + SW], op0=MUL, op1=MUL)
                        nc.vector.scalar_tensor_tensor(
                            p[:, :], v[:, :], invb[:, g - 1:g],
                            Gc[:, o:o + SW], op0=MUL, op1=MUL)
                    else:
                        nc.vector.tensor_mul(
                            q[:, :], ap[:, 0:SW], Mc[:, o:o + SW])
                        nc.vector.tensor_mul(
                            p[:, :], v[:, :], Gc[:, o:o + SW])
                    if t % RENORM == RENORM - 1:
                        nc.vector.tensor_tensor_reduce(
                            an[:, 2:2 + SW], p[:, :], q[:, :],
                            scale=1.0, scalar=0.0,
                            op0=ADD, op1=ADD,
                            accum_out=sums[:, 0:1])
                        nc.vector.reciprocal(invb[:, g:g + 1], sums[:, 0:1])
                    else:
                        nc.vector.tensor_add(
                            an[:, 2:2 + SW], p[:, :], q[:, :])

            for c in range(NCHUNK):
                stream_chunk(c)
                dp_steps(c)

            af = albufs[(T - 1) % 2]
            nc.vector.tensor_add(
                raw_sb[:, 0:1], af[:, 129:130], af[:, 130:131])
            nc.vector.tensor_copy(raw_sb[:, 1:1 + NG], invb[:, 0:NG])
            nc.sync.dma_start(raw_d[:, :], raw_sb[:, :])

    nc.compile()
    return nc


_NC_CACHE = {}


def _make_in_maps(y_true, y_pred):
    in_maps = []
    for k in range(NCORES):
        sl = slice(k * E, (k + 1) * E)
        yt = np.asarray(y_true[sl], dtype=np.int64)
        in_maps.append({
            "y_pred": np.ascontiguousarray(y_pred[sl]),
            "idxw": _pack_idx(yt),
        })
    return in_maps


def _finish(raw):
    """raw [E, 1+NG] -> loss [E, 1]; scales cancel exactly."""
    afin = raw[:, 0].astype(np.float64)
    inv = raw[:, 1:1 + NG].astype(np.float64)
    return (np.log(inv[:, :NG - 1]).sum(1) - np.log(afin))[:, None]


def _numpy_ctc(y_pred, y_true):
    Bn = y_pred.shape[0]
    NEGI = -1e30
    out = np.zeros((Bn, 1), dtype=np.float64)
    logp = np.log(y_pred.astype(np.float64) + EPS)
    for b in range(Bn):
        ext = _build_ext(y_true[b])
        lp = logp[b][:, ext]
        prev2 = np.full(S, -1, dtype=np.int64)
        prev2[2:] = ext[:-2]
        allow = (ext != BLANK) & (ext != prev2)
        al = np.full(S, NEGI)
        al[0], al[1] = lp[0, 0], lp[0, 1]
        for t in range(1, T):
            sh1 = np.concatenate(([NEGI], al[:-1]))
            sh2 = np.where(allow,
                           np.concatenate(([NEGI, NEGI], al[:-2])), NEGI)
            m = np.maximum(np.maximum(al, sh1), sh2)
            al = m + np.log(np.exp(al - m) + np.exp(sh1 - m)
                            + np.exp(sh2 - m)) + lp[t]
        m = max(al[S - 1], al[S - 2])
        out[b, 0] = -(m + np.log(np.exp(al[S - 1] - m)
                                 + np.exp(al[S - 2] - m)))
    return out


def kernel(y_true, y_pred):
    y_true = np.asarray(y_true)
    y_pred = np.ascontiguousarray(np.asarray(y_pred, dtype=np.float32))
    try:
        from concourse.bass_utils import run_bass_kernel_spmd
        if "nc" not in _NC_CACHE:
            _NC_CACHE["nc"] = _build_nc()
        res = run_bass_kernel_spmd(_NC_CACHE["nc"],
                                   _make_in_maps(y_true, y_pred),
                                   core_ids=list(range(NCORES)))
        loss = np.concatenate([_finish(r["raw"]) for r in res.results], 0)
        if not np.all(np.isfinite(loss)):
            raise FloatingPointError("non-finite loss from device")
        return loss.astype(np.float32)
    except Exception:
        return _numpy_ctc(y_pred, y_true).astype(np.float32)
